# revision 1
# baseline (speedup 1.0000x reference)
"""Trainium2 Bass kernel for nn_ConditionedLM (BiLSTM table encoder -> LSTM LM -> vocab decoder).

Strategy (8 NeuronCores, SPMD — one program, per-core data):
  * Host prep: embedding rows are gathered/transposed on the host (pure
    indexing — ~3MB shipped per core instead of the 200MB tables); weights
    are pre-transposed, gate-permuted (i,f,g,o -> i,f,o,g) and cast to bf16.
  * Input projections (x @ Wih.T + b) for all tokens are one big GEMM on
    device, written to DRAM in "strip layout" (row 32j+b) so per-step tiles
    load directly into the gate-PSUM partition layout.
  * LSTM recurrences (encoder fwd+bwd, then the LM) replicated on all
    cores: per-step cost is dominated by streaming Whh through the PE array
    (batch-independent), and an 8-way shard would need a per-step all-gather
    whose latency floor exceeds the whole step.  Replication also means the
    decode needs no communication.
  * Decoder matmul (ys @ Wdec.T + bdec) sharded over vocab: core m computes
    logits[:, :, m*VS:(m+1)*VS].  Three Wdec chunks are preloaded and their
    decode matmuls interleaved into LM-step PE idle gaps.

Per-step structure: gates computed with 4 PE column-group-tiled matmuls
(tile_position=(0,32j)), one per gate strip, concurrent in the 128x128
array (M=B=16 per 32-col strip).  Gate j lands in PSUM partitions
[32j, 32j+16).  The two 512-col halves accumulate in separate PSUM tiles so
the nonlinear chain for cols [0,512) starts as soon as the first half's
k-chunks stop (overlapping the PE's second half-block).  The chain is
chunked and software-pipelined across Activation/Vector/Pool:
gs = pg + xp (DVE, PSUM+SBUF), one sigmoid over partitions 0..79 covers
(i, f, o), tanh over [96,112) covers g, and the cell ops read operands at
co-located partition bases (c state lives at partitions [32,48), tanh(c)
at [64,80)) because the HW requires equal bases for SBUF+SBUF
tensor-tensor ops.  h chunks are re-transposed by the PE as they emerge
and feed the NEXT step's matmuls k-chunk by k-chunk; for the LM the
transposed h goes straight into the ysT buffer that decode uses as lhsT.
The encoder runs fwd and bwd with a half-step offset (they consume
disjoint hT k-chunks), with the bwd cell ops on Pool so the two chains
don't share one engine FIFO.
"""

import numpy as np
import ml_dtypes
from contextlib import ExitStack

import concourse.bass as bass
import concourse.mybir as mybir
import concourse.tile as tile
from concourse import bacc
from concourse.bass_utils import run_bass_kernel_spmd
from concourse.masks import make_identity

dt = mybir.dt
bf16 = ml_dtypes.bfloat16

V, E, He, H = 50257, 512, 512, 1024
B, T, Lt = 16, 128, 64
NCORES = 8
VS = (V + NCORES - 1) // NCORES  # 6283 vocab rows per core (padded)
N_TOK = B * T                    # 2048
N_TAB = B * Lt                   # 1024

MM_DT = dt.bfloat16              # matmul operand dtype (accum stays fp32)
MM_NP = bf16
EW_DT = dt.float32               # elementwise/state dtype

_CACHE = {}
LT_STEPS = Lt
T_STEPS = T
DEC_NC = None
DEC_PRE = 5                      # Wdec chunks preloaded + decoded inside LM
# nonlinearity pipeline chunks (col ranges): small first chunks minimize
# the latency to the first transposed h (the PE's step-head dependency)
CHUNKS = [(0, 256), (256, 512), (512, 768), (768, 1024)]
ACT = mybir.ActivationFunctionType


def _gate_perm(h):
    # torch gate order i,f,g,o -> i,f,o,g
    return np.concatenate([np.arange(0, h), np.arange(h, 2 * h),
                           np.arange(3 * h, 4 * h), np.arange(2 * h, 3 * h)])


def _ceil_div(a, b):
    return (a + b - 1) // b


def _bcast_ap(dram_tensor, n_free):
    """AP reading dram_tensor's single row broadcast to 128 partitions."""
    return bass.AP(dram_tensor, 0, [[0, 128], [1, n_free]])


def build_bass():
    nc = bacc.Bacc()

    embT_d = nc.dram_tensor("embT", [128, (E // 128) * N_TOK], MM_DT,
                            kind="ExternalInput")
    tembT_d = nc.dram_tensor("tembT", [128, (E // 128) * N_TAB], MM_DT,
                             kind="ExternalInput")
    wih_enc_d = nc.dram_tensor("wih_enc_t", [E, 8 * He], MM_DT, kind="ExternalInput")
    wih_lm_d = nc.dram_tensor("wih_lm_t", [E, 4 * H], MM_DT, kind="ExternalInput")
    whh_f_d = nc.dram_tensor("whh_f_t", [He, 4 * He], MM_DT, kind="ExternalInput")
    whh_b_d = nc.dram_tensor("whh_b_t", [He, 4 * He], MM_DT, kind="ExternalInput")
    whh_lm_d = nc.dram_tensor("whh_lm_t", [H, 4 * H], MM_DT, kind="ExternalInput")
    b_enc_d = nc.dram_tensor("b_enc", [1, 8 * He], dt.float32, kind="ExternalInput")
    b_lm_d = nc.dram_tensor("b_lm", [1, 4 * H], dt.float32, kind="ExternalInput")
    wdec_d = nc.dram_tensor("wdec_t", [H, VS], MM_DT, kind="ExternalInput")
    bdec_d = nc.dram_tensor("bdec_s", [1, VS], dt.float32, kind="ExternalInput")
    h0_d = nc.dram_tensor("enc_h0", [2, B, He], dt.float32, kind="ExternalInput")
    c0_d = nc.dram_tensor("enc_c0", [2, B, He], dt.float32, kind="ExternalInput")
    out_d = nc.dram_tensor("out", [N_TOK, VS], dt.float32, kind="ExternalOutput")

    with tile.TileContext(nc) as tc, ExitStack() as ctx:
        # DRAM intermediates for input projections, staged in "strip layout":
        # dim1 index 32j+b = PSUM gate-strip partition, so the per-step tile
        # loads straight into the layout the chain's add expects.  Rows
        # 32j+16..32j+31 are never written (read as don't-care).
        dram = ctx.enter_context(tc.tile_pool(name="dram", bufs=1, space="DRAM"))
        xp_lm_d = dram.tile([T, 128, 1024], MM_DT)
        xf_enc_d = dram.tile([Lt, 128, 512], MM_DT)
        xb_enc_d = dram.tile([Lt, 128, 512], MM_DT)

        const = ctx.enter_context(tc.tile_pool(name="const", bufs=1))
        ident = const.tile([16, 16], dt.float32)
        make_identity(nc, ident[:])

        psum_mm = ctx.enter_context(
            tc.tile_pool(name="psum_mm", bufs=2, space="PSUM"))
        ysT_p = ctx.enter_context(tc.tile_pool(name="ysT", bufs=1))
        ysT = ysT_p.tile([128, 8, N_TOK], MM_DT)
        wdec_pp = ctx.enter_context(tc.tile_pool(name="wdec_pp", bufs=1))

        # =========================================================
        # Phase A+B: gather + transpose embeddings, input projections
        # =========================================================
        with tc.tile_pool(name="gather", bufs=1) as gpool, \
             tc.tile_pool(name="bias", bufs=2) as bpool, \
             tc.tile_pool(name="wih", bufs=7) as wpool, \
             tc.tile_pool(name="xpout", bufs=2) as xpo:
            bias_bcs = []
            for bias_src in (b_enc_d, b_lm_d):
                bias_bc = bpool.tile([128, 4096], dt.float32, tag="bbc")
                nc.sync.dma_start(bias_bc[:], _bcast_ap(bias_src, 4096))
                bias_bcs.append(bias_bc)

            lm_base = xp_lm_d[:, :, :]
            f_base = xf_enc_d[:, :, :]
            b_base = xb_enc_d[:, :, :]
            # host-gathered transposed embeddings -> SBUF, then project
            embT = gpool.tile([128, E // 128, N_TOK], MM_DT)
            tembT = gpool.tile([128, E // 128, N_TAB], MM_DT)
            nc.sync.dma_start(tembT[:], tembT_d[:])
            for k in range(E // 128):
                nc.sync.dma_start(
                    embT[:, k, :], embT_d[:, N_TOK * k:N_TOK * (k + 1)])
            for i, (eT, n_rows, wih_src) in enumerate(
                    ((tembT, N_TAB, wih_enc_d), (embT, N_TOK, wih_lm_d))):
                is_enc = (i == 0)
                bias_bc = bias_bcs[0 if is_enc else 1]
                wchunks = []
                for k in range(4):
                    wc = wpool.tile([128, 4096], MM_DT, tag="wih")
                    nc.scalar.dma_start(wc[:], wih_src[128 * k:128 * (k + 1), :])
                    wchunks.append(wc)
                for g in range(n_rows // 128):
                    xog = xpo.tile([128, 4096], MM_DT, tag="xo")
                    for n in range(8):
                        px = psum_mm.tile([128, 512], dt.float32, tag="mm")
                        for k in range(4):
                            nc.tensor.matmul(
                                px[:], eT[:, k, 128 * g:128 * (g + 1)],
                                wchunks[k][:, 512 * n:512 * (n + 1)],
                                start=(k == 0), stop=(k == 3))
                        nc.vector.tensor_add(
                            xog[:, 512 * n:512 * (n + 1)], px[:],
                            bias_bc[:, 512 * n:512 * (n + 1)])
                    # batched scatter to strip layout: src partition p=16dt+b
                    # -> dst [step 8g+dt, strip-row 32j+b, units]; one DMA
                    # per gate strip j (3-dim APs balance against the SBUF
                    # source), issued on alternating queues
                    eng = nc.sync if g % 2 == 0 else nc.scalar
                    if is_enc:
                        for base, half in ((f_base, 0), (b_base, 1)):
                            for j in range(4):
                                ap = bass.AP(
                                    base.tensor,
                                    base.offset + g * 8 * 65536 + 32 * j * 512,
                                    [[65536, 8], [512, 16], [1, 512]])
                                eng.dma_start(
                                    ap, xog[:, 2048 * half + 512 * j:
                                            2048 * half + 512 * (j + 1)])
                    else:
                        for j in range(4):
                            ap = bass.AP(
                                lm_base.tensor,
                                lm_base.offset + g * 8 * 131072 + 32 * j * 1024,
                                [[131072, 8], [1024, 16], [1, 1024]])
                            eng.dma_start(ap, xog[:, 1024 * j:1024 * (j + 1)])

        # =========================================================
        # Recurrences
        # =========================================================
        with tc.tile_pool(name="state", bufs=2) as state, \
             tc.tile_pool(name="one", bufs=1) as one_p, \
             tc.tile_pool(name="sig", bufs=2) as sig_p, \
             tc.tile_pool(name="tmp", bufs=2) as tmp_p, \
             tc.tile_pool(name="xp", bufs=2) as xp_p, \
             tc.tile_pool(name="psum_g", bufs=2, space="PSUM") as psum_g, \
             tc.tile_pool(name="psum_h", bufs=2, space="PSUM") as psum_h:

            NCH = len(CHUNKS)         # chain chunks per step

            # The HW requires equal partition bases when both inputs of a
            # tensor_tensor op live in SBUF, so operands are co-located:
            # the c state sits at partitions [32,48) (pairs with sigmoid(f)
            # at sigb[32:48]) and tanh(c) at [64,80) (pairs with sigmoid(o)).
            def chain_alloc():
                """Per-step tiles for the nonlinear chain."""
                sigb = sig_p.tile([80, 1024], EW_DT, tag="sigb")
                tg = sig_p.tile([16, 1024], EW_DT, tag="tg")
                t1 = tmp_p.tile([16, 1024], EW_DT, tag="t1")
                t2 = tmp_p.tile([16, 1024], EW_DT, tag="t2")
                tcn = one_p.tile([80, 1024], EW_DT, tag="tc")
                c_new = state.tile([48, 1024], EW_DT, tag="c")
                h_new = state.tile([16, 1024], EW_DT, tag="h")
                return sigb, tg, t1, t2, tcn, c_new, h_new

            def chain_alloc_gs():
                gs = one_p.tile([112, 1024], EW_DT, tag="gs", name="gs")
                return gs

            def chain_emit(ch, gs, halves, c_prev):
                """Emit the nonlinear chain for one step, software-pipelined
                per chunk so the first h chunk emerges with minimal latency
                and no engine FIFO head-blocks.  halves = [(psum_half, xp_ap)]
                for gate cols [0,512) and [512,1024); the xp add happens here
                (gs = pg + xp) instead of as PE identity matmuls."""
                sigb, tg, t1, t2, tcn, c_new, h_new = ch

                def srcs(c):
                    lo, hi = CHUNKS[c]
                    pg, xp = halves[0 if lo < 512 else 1]
                    off = 0 if lo < 512 else 512
                    return pg, xp, slice(lo - off, hi - off)

                sl = [slice(lo, hi) for lo, hi in CHUNKS]
                for c in range(NCH + 2):
                    if c < NCH:
                        pg, xp, ps = srcs(c)
                        s = sl[c]
                        nc.vector.tensor_add(gs[:, s], pg[0:112, ps],
                                             xp[0:112, ps])
                    if 1 <= c <= NCH:
                        s = sl[c - 1]
                        nc.scalar.activation(sigb[:, s], gs[0:80, s], ACT.Sigmoid)
                        nc.scalar.activation(tg[:, s], gs[96:112, s], ACT.Tanh)
                        nc.vector.tensor_mul(t1[:, s], sigb[32:48, s],
                                             c_prev[32:48, s])
                        nc.gpsimd.tensor_mul(t2[:, s], sigb[0:16, s], tg[:, s])
                        nc.vector.tensor_add(c_new[32:48, s], t1[:, s], t2[:, s])
                    if c >= 2:
                        s = sl[c - 2]
                        nc.scalar.activation(tcn[64:80, s], c_new[32:48, s],
                                             ACT.Tanh)
                        nc.vector.tensor_mul(h_new[:, s], sigb[64:80, s],
                                             tcn[64:80, s])

            def chain_trans(ch, ph, hT_chunk_of):
                """Transpose h chunk c -> psum -> hT slices (per 128-col k)."""
                sigb, tg, t1, t2, tcn, c_new, h_new = ch

                def do(c):
                    k0, k1 = CHUNKS[c][0] // 128, CHUNKS[c][1] // 128
                    for kk in range(k0, k1):
                        nc.tensor.transpose(ph[:, kk, :],
                                            h_new[:, 128 * kk:128 * (kk + 1)],
                                            ident[:])
                    nc.vector.tensor_copy(
                        hT_chunk_of(k0, k1 - k0), ph[:, k0:k1, :])
                return do

            # ---- Phase C: encoder (fwd + bwd fused), 64 steps ----
            whh_enc_ctx = tc.tile_pool(name="whh_enc", bufs=1)
            whh_enc_p = whh_enc_ctx.__enter__()
            whh_f_sb = whh_enc_p.tile([128, 4, 4 * He], MM_DT, tag="wenc_f")
            whh_b_sb = whh_enc_p.tile([128, 4, 4 * He], MM_DT, tag="wenc_b")
            for k in range(4):
                nc.sync.dma_start(whh_f_sb[:, k, :], whh_f_d[128 * k:128 * (k + 1), :])
                nc.sync.dma_start(whh_b_sb[:, k, :], whh_b_d[128 * k:128 * (k + 1), :])

            h_cur = state.tile([16, 1024], EW_DT, tag="h")
            c_cur = state.tile([48, 1024], EW_DT, tag="c")
            nc.sync.dma_start(h_cur[:, 0:512], h0_d[0])
            nc.sync.dma_start(h_cur[:, 512:1024], h0_d[1])
            nc.sync.dma_start(c_cur[32:48, 0:512], c0_d[0])
            nc.sync.dma_start(c_cur[32:48, 512:1024], c0_d[1])
            hT = state.tile([128, 8, 16], MM_DT, tag="hT")
            ph0 = psum_h.tile([128, 8, 16], dt.float32, tag="ph")
            for k in range(8):
                nc.tensor.transpose(ph0[:, k, :],
                                    h_cur[:, 128 * k:128 * (k + 1)],
                                    ident[:])
            nc.vector.tensor_copy(hT[:], ph0[:])

            def enc_xp(s):
                """DMA strip-layout x-projections for encoder step s."""
                xf = xp_p.tile([128, 512], MM_DT, tag="xf")
                nc.sync.dma_start(xf[:], xf_enc_d[s])
                xb = xp_p.tile([128, 512], MM_DT, tag="xb")
                nc.sync.dma_start(xb[:], xb_enc_d[Lt - 1 - s])
                return xf, xb

            def enc_gates_k(pg, hT_k_ap, whh_sb, k, stop):
                """One k-chunk of the encoder gates matmul into half-tile pg."""
                for j in range(4):
                    nc.tensor.matmul(
                        pg[32 * j:32 * j + 16, :],
                        hT_k_ap, whh_sb[:, k, 512 * j:512 * (j + 1)],
                        start=(k == 0), stop=stop,
                        tile_position=(0, 32 * j))

            # Half-step-offset pipeline: fwd and bwd gates use disjoint
            # hT chunks (K splits), so the PE computes one direction's gates
            # while the other direction's nonlinear chain runs.  Emission
            # order per step s:
            #   chain-fwd(s) | trans-bwd(s-1) + bwd-gates(s) |
            #   chain-bwd(s) | trans-fwd(s) + fwd-gates(s+1)
            def chain_half(ch, gs, pg, xp, half, c_prev):
                # fwd cascade on DVE, bwd cell ops on Pool, so the two
                # directions' chains don't serialize through one engine
                # FIFO (the gs add reads PSUM, so it must stay on DVE)
                sigb, tg, t1, t2, tcn, c_new, h_new = ch
                if half == 0:
                    e_t1, e_t2, e_addc, e_h = (nc.vector, nc.gpsimd,
                                               nc.vector, nc.vector)
                else:
                    e_t1, e_t2, e_addc, e_h = (nc.gpsimd, nc.vector,
                                               nc.gpsimd, nc.gpsimd)
                cs = [c for c in range(NCH)
                      if (CHUNKS[c][0] < 512) == (half == 0)]
                off = 0 if half == 0 else 512
                for c in cs:
                    lo, hi = CHUNKS[c]
                    s_ = slice(lo, hi)
                    ps = slice(lo - off, hi - off)
                    nc.vector.tensor_add(gs[:, s_], pg[0:112, ps],
                                         xp[0:112, ps])
                for c in cs:
                    s_ = slice(*CHUNKS[c])
                    nc.scalar.activation(sigb[:, s_], gs[0:80, s_], ACT.Sigmoid)
                    nc.scalar.activation(tg[:, s_], gs[96:112, s_], ACT.Tanh)
                    e_t1.tensor_mul(t1[:, s_], sigb[32:48, s_],
                                    c_prev[32:48, s_])
                    e_t2.tensor_mul(t2[:, s_], sigb[0:16, s_], tg[:, s_])
                    e_addc.tensor_add(c_new[32:48, s_], t1[:, s_], t2[:, s_])
                for c in cs:
                    s_ = slice(*CHUNKS[c])
                    nc.scalar.activation(tcn[64:80, s_], c_new[32:48, s_],
                                         ACT.Tanh)
                    e_h.tensor_mul(h_new[:, s_], sigb[64:80, s_],
                                   tcn[64:80, s_])

            # bootstrap: fwd gates of step 0 from hT(init)
            pga = psum_g.tile([128, 512], dt.float32, tag="pga")
            xfb = enc_xp(0)
            for k in range(4):
                enc_gates_k(pga, hT[:, k, :], whh_f_sb, k, k == 3)

            hT_prev, ph_prev, trans_prev = hT, None, None
            for s in range(LT_STEPS):
                last = (s == LT_STEPS - 1)
                ch = chain_alloc()
                gs = chain_alloc_gs()
                # 1) fwd chain of step s (pga(s) complete)
                chain_half(ch, gs, pga, xfb[0], 0, c_cur)
                # 2) bwd gates of step s, consuming hT-bwd(s-1) as the
                #    previous step's bwd transposes land
                pgb = psum_g.tile([128, 512], dt.float32, tag="pgb")
                if s == 0:
                    for k in range(4):
                        enc_gates_k(pgb, hT_prev[:, 4 + k, :], whh_b_sb, k,
                                    k == 3)
                else:
                    bwd_cs = [c for c in range(NCH) if CHUNKS[c][0] >= 512]
                    for c in bwd_cs:
                        trans_prev(c)
                    for c in bwd_cs:
                        for kk in range(CHUNKS[c][0] // 128,
                                        CHUNKS[c][1] // 128):
                            enc_gates_k(pgb, hT_prev[:, kk, :], whh_b_sb,
                                        kk - 4, kk == 7)
                # 3) bwd chain of step s
                chain_half(ch, gs, pgb, xfb[1], 1, c_cur)
                # 4) fwd gates of step s+1, consuming hT-fwd(s)
                if not last:
                    xfb = enc_xp(s + 1)
                    hT = state.tile([128, 8, 16], MM_DT, tag="hT")
                    ph = psum_h.tile([128, 8, 16], dt.float32, tag="ph")
                    trans = chain_trans(ch, ph, lambda k0, n: hT[:, k0:k0 + n, :])
                    pga = psum_g.tile([128, 512], dt.float32, tag="pga")
                    fwd_cs = [c for c in range(NCH) if CHUNKS[c][0] < 512]
                    for c in fwd_cs:
                        trans(c)
                    for c in fwd_cs:
                        for kk in range(CHUNKS[c][0] // 128,
                                        CHUNKS[c][1] // 128):
                            enc_gates_k(pga, hT[:, kk, :], whh_f_sb, kk,
                                        kk == 3)
                    hT_prev, ph_prev, trans_prev = hT, ph, trans
                c_cur = ch[5]
                h_cur = ch[6]

            # ---- Phase D: reshape final states -> LM initial state ----
            h_lm = state.tile([16, 1024], EW_DT, tag="h")
            c_lm = state.tile([48, 1024], EW_DT, tag="c")
            # h_lm row r<8:  [src[2r, fwd], src[2r+1, fwd]]
            # h_lm row r>=8: [src[2(r-8), bwd], src[2(r-8)+1, bwd]]
            for dst, src, p0 in ((h_lm, h_cur, 0), (c_lm, c_cur, 32)):
                for rh in range(2):         # 0: fwd rows (r<8), 1: bwd rows
                    for chh in range(2):    # dest col half = even/odd src row
                        nc.sync.dma_start(
                            dst[p0 + 8 * rh:p0 + 8 * rh + 8,
                                512 * chh:512 * chh + 512],
                            src[p0 + chh:p0 + 16:2,
                                512 * rh:512 * rh + 512])
            hT = state.tile([128, 8, 16], MM_DT, tag="hT")
            ph0 = psum_h.tile([128, 8, 16], dt.float32, tag="ph")
            for k in range(8):
                nc.tensor.transpose(ph0[:, k, :],
                                    h_lm[:, 128 * k:128 * (k + 1)],
                                    ident[:])
            nc.vector.tensor_copy(hT[:], ph0[:])
            c_cur = c_lm

            whh_enc_ctx.__exit__(None, None, None)

            # ---- Phase E: LM recurrence, 128 steps ----
            whh_lm_ctx = tc.tile_pool(name="whh_lm", bufs=1)
            whh_lm_p = whh_lm_ctx.__enter__()
            whh_sb = whh_lm_p.tile([128, 8, 4 * H], MM_DT, tag="wlm")
            for k in range(8):
                (nc.sync if k % 2 == 0 else nc.scalar).dma_start(
                    whh_sb[:, k, :], whh_lm_d[128 * k:128 * (k + 1), :])
            # preload DEC_PRE vocab chunks of Wdec; their (n, m) decode units
            # are interleaved into LM-step PE idle gaps
            wn_pre = wdec_pp.tile([128, DEC_PRE, 8, 512], MM_DT)
            for n in range(DEC_PRE):
                for k in range(8):
                    nc.scalar.dma_start(
                        wn_pre[:, n, k, :],
                        wdec_d[128 * k:128 * (k + 1), 512 * n:512 * (n + 1)])
            bias_pre = wdec_pp.tile([128, DEC_PRE * 512], MM_DT)
            nc.gpsimd.dma_start(bias_pre[:], _bcast_ap(bdec_d, DEC_PRE * 512))
            dec_units = []           # (n, m) units decoded during the LM
            dec_done = set()

            def emit_dec_unit(n, m):
                pd = psum_mm.tile([128, 512], dt.float32, tag="mm")
                for k in range(8):
                    nc.tensor.matmul(
                        pd[:], ysT[:, k, 128 * m:128 * (m + 1)],
                        wn_pre[:, n, k, :], start=(k == 0), stop=(k == 7))
                ob = xp_p.tile([128, 512], dt.float32, tag="ob")
                nc.vector.tensor_add(ob[:], pd[:],
                                     bias_pre[:, 512 * n:512 * (n + 1)])
                nc.sync.dma_start(
                    out_d[128 * m:128 * (m + 1), 512 * n:512 * (n + 1)],
                    ob[:])
                dec_done.add((n, m))

            def lm_xp(t):
                xt = xp_p.tile([128, 1024], MM_DT, tag="xf")
                nc.sync.dma_start(xt[:], xp_lm_d[t])
                return xt

            def lm_gates_k(pg, hT_k_ap, k, h2, stop):
                for j in range(4):
                    nc.tensor.matmul(
                        pg[32 * j:32 * j + 16, :],
                        hT_k_ap,
                        whh_sb[:, k, 1024 * j + 512 * h2:
                               1024 * j + 512 * (h2 + 1)],
                        start=(k == 0), stop=stop, tile_position=(0, 32 * j))

            # bootstrap: step 0 gates from hT(init); h2-major
            pga = psum_g.tile([128, 512], dt.float32, tag="pga")
            pgb = psum_g.tile([128, 512], dt.float32, tag="pgb")
            xt_c = lm_xp(0)
            for h2, pg in ((0, pga), (1, pgb)):
                for k in range(8):
                    lm_gates_k(pg, hT[:, k, :], k, h2, k == 7)

            for t in range(T_STEPS):
                last = (t == T_STEPS - 1)
                ch = chain_alloc()
                sigb, tg, t1, t2, tcn, c_new, h_new = ch
                gs = chain_alloc_gs()
                ph = psum_h.tile([128, 8, 16], dt.float32, tag="ph")
                trans = chain_trans(
                    ch, ph,
                    lambda k0, n, t=t: ysT[:, k0:k0 + n, 16 * t:16 * t + 16])
                if not last:
                    pga_n = psum_g.tile([128, 512], dt.float32, tag="pga")
                    pgb_n = psum_g.tile([128, 512], dt.float32, tag="pgb")
                    xt_n = lm_xp(t + 1)
                if t % 2 == 0 and dec_units:
                    emit_dec_unit(*dec_units.pop(0))
                if t % 8 == 7 and t < T_STEPS - 1:
                    m_ready = t // 8
                    dec_units.extend((n, m_ready) for n in range(DEC_PRE))
                chain_emit(ch, gs,
                           [(pga, xt_c[:, 0:512]), (pgb, xt_c[:, 512:1024])],
                           c_cur)
                # h2=0 of next step's gates interleaved with transposes,
                # h2=1 afterwards (all hT chunks then ready)
                trans(0)
                for c in range(NCH):
                    if c + 1 < NCH:
                        trans(c + 1)
                    if not last:
                        for kk in range(CHUNKS[c][0] // 128,
                                        CHUNKS[c][1] // 128):
                            lm_gates_k(pga_n,
                                       ysT[:, kk, 16 * t:16 * t + 16],
                                       kk, 0, kk == 7)
                if not last:
                    for kk in range(8):
                        lm_gates_k(pgb_n, ysT[:, kk, 16 * t:16 * t + 16],
                                   kk, 1, kk == 7)
                c_cur = c_new
                if not last:
                    pga, pgb = pga_n, pgb_n
                    xt_c = xt_n
            whh_lm_ctx.__exit__(None, None, None)

        # =========================================================
        # Phase F: decode (vocab-sharded): out = ysT.T @ WdecT + bdec
        # =========================================================
        with tc.tile_pool(name="wdec", bufs=3) as wdp, \
             tc.tile_pool(name="dbias", bufs=1) as dbp, \
             tc.tile_pool(name="dout", bufs=6) as dop:
            n_nc = DEC_NC or _ceil_div(VS, 512)
            bias_dec = dbp.tile([128, VS], dt.float32)
            nc.sync.dma_start(bias_dec[:], _bcast_ap(bdec_d, VS))

            for n in range(n_nc):
                nw = min(512, VS - 512 * n)
                if n < DEC_PRE:
                    wn = wn_pre[:, n, :, :]
                else:
                    wnt = wdp.tile([128, 8, 512], MM_DT, tag="wn")
                    for k in range(8):
                        nc.sync.dma_start(
                            wnt[:, k, :nw],
                            wdec_d[128 * k:128 * (k + 1), 512 * n:512 * n + nw])
                    wn = wnt[:, :, :]
                for m in range(N_TOK // 128):
                    if (n, m) in dec_done:
                        continue
                    pd = psum_mm.tile([128, 512], dt.float32, tag="mm")
                    for k in range(8):
                        nc.tensor.matmul(
                            pd[:, :nw], ysT[:, k, 128 * m:128 * (m + 1)],
                            wn[:, k, :nw], start=(k == 0), stop=(k == 7))
                    ob = dop.tile([128, 512], dt.float32, tag="ob")
                    nc.vector.tensor_add(ob[:, :nw], pd[:, :nw],
                                         bias_dec[:, 512 * n:512 * n + nw])
                    nc.sync.dma_start(
                        out_d[128 * m:128 * (m + 1), 512 * n:512 * n + nw],
                        ob[:, :nw])

    nc.compile()
    return nc


def _embT_host(tbl, idx):
    """Gather embedding rows for flat token order r=16t+b and lay out as
    [128 partitions, (E//128) * n_rows] (the transposed lhsT layout)."""
    g = np.asarray(tbl, np.float32)[idx]            # [n, E]
    n = g.shape[0]
    gt = g.T.reshape(E // 128, 128, n)              # [k, p, n]
    return np.ascontiguousarray(
        gt.transpose(1, 0, 2).reshape(128, -1)).astype(MM_NP)


def _prep_inputs(inputs):
    f32 = np.float32
    x = np.asarray(inputs["x"]).astype(np.int64)
    table = np.asarray(inputs["table"]).astype(np.int64)
    xf = x.T.reshape(-1)        # row r = 16t+b
    tf = table.T.reshape(-1)

    pe = _gate_perm(He)
    pl = _gate_perm(H)
    wih_enc_t = np.concatenate(
        [np.asarray(inputs["Wih_f"])[pe].T, np.asarray(inputs["Wih_b"])[pe].T],
        axis=1).astype(MM_NP)                       # [512, 4096]
    b_enc = np.concatenate(
        [np.asarray(inputs["b_f"])[pe], np.asarray(inputs["b_b"])[pe]])[None]
    wih_lm_t = np.ascontiguousarray(np.asarray(inputs["Wih_lm"])[pl].T).astype(MM_NP)
    whh_f_t = np.ascontiguousarray(np.asarray(inputs["Whh_f"])[pe].T).astype(MM_NP)
    whh_b_t = np.ascontiguousarray(np.asarray(inputs["Whh_b"])[pe].T).astype(MM_NP)
    whh_lm_t = np.ascontiguousarray(np.asarray(inputs["Whh_lm"])[pl].T).astype(MM_NP)

    wdec = np.asarray(inputs["Wdec"]).astype(f32)
    bdec = np.asarray(inputs["bdec"]).astype(f32)
    wdec_pad = np.zeros((NCORES * VS, H), f32)
    wdec_pad[:V] = wdec
    bdec_pad = np.zeros(NCORES * VS, f32)
    bdec_pad[:V] = bdec

    common = dict(
        embT=_embT_host(inputs["embed"], xf),
        tembT=_embT_host(inputs["table_embed"], tf),
        wih_enc_t=wih_enc_t, wih_lm_t=wih_lm_t,
        whh_f_t=whh_f_t, whh_b_t=whh_b_t, whh_lm_t=whh_lm_t,
        b_enc=b_enc.astype(f32),
        b_lm=np.asarray(inputs["b_lm"])[pl][None].astype(f32),
        enc_h0=np.asarray(inputs["enc_h0"], f32),
        enc_c0=np.asarray(inputs["enc_c0"], f32),
    )
    in_maps = []
    for c in range(NCORES):
        m = dict(common)
        m["wdec_t"] = np.ascontiguousarray(
            wdec_pad[c * VS:(c + 1) * VS].T).astype(MM_NP)
        m["bdec_s"] = np.ascontiguousarray(bdec_pad[None, c * VS:(c + 1) * VS])
        in_maps.append(m)
    return in_maps


def kernel(**inputs) -> np.ndarray:
    import time as _time
    if "nc" not in _CACHE:
        _CACHE["nc"] = build_bass()
    nc = _CACHE["nc"]
    in_maps = _prep_inputs(inputs)
    res = None
    for attempt in range(3):
        try:
            res = run_bass_kernel_spmd(nc, in_maps, core_ids=list(range(NCORES)))
            break
        except Exception:
            # transient NRT_EXEC_UNIT_UNRECOVERABLE has been observed right
            # after a crashed predecessor session; back off and retry
            if attempt == 2:
                raise
            _time.sleep(10)
    outs = [res.results[c]["out"] for c in range(NCORES)]
    full = np.concatenate(outs, axis=1)[:, :V]       # [2048, 50257]
    return np.ascontiguousarray(full.reshape(T, B, V))


if __name__ == "__main__":
    nc = build_bass()
    print("build ok")



# revision 6
# speedup vs baseline: 1.8971x; 1.8971x over previous
"""Trainium2 Bass kernel for nn_ConditionedLM (BiLSTM table encoder -> LSTM LM -> vocab decoder).

Strategy (8 NeuronCores, SPMD — one program, per-core data):
  * Fully transposed ("feature-major") layout: features live on SBUF/PSUM
    partitions and the batch (B=16) lives on the free dimension.  Gate
    pre-activations are computed as gates.T[strip, b] = Whh.T-chunk.T @ hT
    with the weight chunk as the 128x128 stationary operand and the tiny
    [128,16] hT slice as the moving operand, so each matmul moves only 16
    columns.  The hidden state is *already* transposed for the next step's
    matmul (and for the decoder), eliminating every PE transpose, the hT
    copies, and the strip-scatter of the old layout.
  * Elementwise chain also runs feature-major: sigmoid/tanh/cell ops span
    128 partitions x (chunk, batch) free cols, and all elementwise math is
    bf16 (validated ~7e-3 rel err vs the 2e-2 gate).
  * Input projections x @ Wih.T are batched GEMMs (tokens on the moving
    dim), bias is folded in by the Activation engine (per-partition bias in
    this orientation), and per-step slices land in DRAM as [step][128][
    strip,b] tiles that a single identity matmul injects into the gate
    PSUM accumulation (no DVE add).
  * Encoder fwd+bwd share one PSUM tile per step (fwd cols 0:256, bwd
    256:512), one LSTM-cell chain each, interleaved.
  * Decoder is vocab-sharded across cores; logits are produced vocab-major
    ([128 vocab rows, 512 tokens] PSUM tiles), bias-added on the Activation
    engine (per-partition again), written bf16.
  * LM recurrence is replicated on all cores (it is batch-small and
    latency-bound; sharding it would need a per-step all-gather).
"""

import numpy as np
import ml_dtypes
from contextlib import ExitStack

import concourse.bass as bass
import concourse.mybir as mybir
import concourse.tile as tile
from concourse import bacc
from concourse.bass_utils import run_bass_kernel_spmd
from concourse.masks import make_identity

dt = mybir.dt
bf16 = ml_dtypes.bfloat16

V, E, He, H = 50257, 512, 512, 1024
B, T, Lt = 16, 128, 64
NCORES = 8
VSTRIPS = 50                     # vocab strips of 128 per core
VS = VSTRIPS * 128               # 6400 padded vocab rows per core
N_TOK = B * T                    # 2048
N_TAB = B * Lt                   # 1024

MM_DT = dt.bfloat16
MM_NP = bf16
ACT = mybir.ActivationFunctionType

_CACHE = {}


def _gate_perm(h):
    # torch gate order i,f,g,o -> i,f,o,g
    return np.concatenate([np.arange(0, h), np.arange(h, 2 * h),
                           np.arange(3 * h, 4 * h), np.arange(2 * h, 3 * h)])


def build_bass():
    nc = bacc.Bacc()

    embT_d = nc.dram_tensor("embT", [128, (E // 128) * N_TOK], MM_DT,
                            kind="ExternalInput")
    tembT_d = nc.dram_tensor("tembT", [128, (E // 128) * N_TAB], MM_DT,
                             kind="ExternalInput")
    wih_enc_d = nc.dram_tensor("wih_enc_t", [E, 4096], MM_DT, kind="ExternalInput")
    wih_lm_d = nc.dram_tensor("wih_lm_t", [E, 4096], MM_DT, kind="ExternalInput")
    whh_enc_d = nc.dram_tensor("whh_enc_t", [He, 4096], MM_DT, kind="ExternalInput")
    whh_lm_d = nc.dram_tensor("whh_lm_t", [H, 4096], MM_DT, kind="ExternalInput")
    b_enc_d = nc.dram_tensor("b_enc_s", [128, 32], dt.float32, kind="ExternalInput")
    b_lm_d = nc.dram_tensor("b_lm_s", [128, 32], dt.float32, kind="ExternalInput")
    h0T_d = nc.dram_tensor("h0T", [128, 128], MM_DT, kind="ExternalInput")
    c0T_d = nc.dram_tensor("c0T", [128, 128], MM_DT, kind="ExternalInput")
    wdec_d = nc.dram_tensor("wdec_t", [H, VS], MM_DT, kind="ExternalInput")
    bdec_d = nc.dram_tensor("bdec_s", [128, VSTRIPS], dt.float32,
                            kind="ExternalInput")
    out_d = nc.dram_tensor("out", [VS, N_TOK], MM_DT, kind="ExternalOutput")

    with tile.TileContext(nc) as tc, ExitStack() as ctx:
        dram = ctx.enter_context(tc.tile_pool(name="dram", bufs=1, space="DRAM"))
        # per-step projection tiles: [step][partition=row-in-strip][16s+b]
        xp_enc_d = dram.tile([Lt, 128, 512], MM_DT)
        xp_lm_d = dram.tile([T, 128, 512], MM_DT)

        const = ctx.enter_context(tc.tile_pool(name="const", bufs=1))
        ident = const.tile([128, 128], MM_DT)
        make_identity(nc, ident[:])
        bias_p = ctx.enter_context(tc.tile_pool(name="bias", bufs=1))
        b_enc_sb = bias_p.tile([128, 32], dt.float32)
        b_lm_sb = bias_p.tile([128, 32], dt.float32)
        bdec_sb = bias_p.tile([128, VSTRIPS], dt.float32)
        nc.sync.dma_start(b_enc_sb[:], b_enc_d[:])
        nc.sync.dma_start(b_lm_sb[:], b_lm_d[:])
        nc.sync.dma_start(bdec_sb[:], bdec_d[:])

        ysT_p = ctx.enter_context(tc.tile_pool(name="ysT", bufs=1))
        # slot t+1 holds ys[t]; slot 0 is the LM initial state
        ysT = ysT_p.tile([128, 8, 16 * (T + 1)], MM_DT)
        whh_lm_p = ctx.enter_context(tc.tile_pool(name="whh_lm", bufs=1))
        whh_lm = whh_lm_p.tile([128, 8, 4096], MM_DT)
        for k in range(8):
            nc.gpsimd.dma_start(whh_lm[:, k, :], whh_lm_d[128 * k:128 * (k + 1), :])

        psum_mm = ctx.enter_context(
            tc.tile_pool(name="psum_mm", bufs=2, space="PSUM"))

        # =========================================================
        # Phase A/B: input projections (tokens on the moving dim)
        # =========================================================
        # xp[t][p][16s+b]: strip s pre-activation rows 128s..128s+128 of the
        # (i,f,o,g | fwd,bwd-interleaved) gate layout, batch b.
        def proj_unit(wih_sb, eT_sb, nk, s, blk, bias_sb, xp_base, xpo):
            px = psum_mm.tile([128, 512], dt.float32, tag="mm")
            for k in range(nk):
                nc.tensor.matmul(
                    px[:], wih_sb[:, k, 128 * s:128 * (s + 1)],
                    eT_sb[:, k, 512 * blk:512 * (blk + 1)],
                    start=(k == 0), stop=(k == nk - 1))
            xog = xpo.tile([128, 512], MM_DT, tag="xo")
            nc.scalar.activation(xog[:], px[:], ACT.Identity,
                                 bias=bias_sb[:, s:s + 1])
            # scatter: src col 16*tt+b -> dst [blk*32+tt][p][16s+b]
            ap = bass.AP(xp_base.tensor,
                         xp_base.offset + blk * 32 * 65536 + 16 * s,
                         [[512, 128], [65536, 32], [1, 16]])
            nc.scalar.dma_start(ap, xog[:])

        with tc.tile_pool(name="gather", bufs=1) as gpool, \
             tc.tile_pool(name="wih", bufs=1) as wpool, \
             tc.tile_pool(name="xpout", bufs=4) as xpo:
            tembT = gpool.tile([128, 4, N_TAB], MM_DT)
            embT = gpool.tile([128, 4, N_TOK], MM_DT)
            nc.sync.dma_start(tembT[:], tembT_d[:])
            for k in range(4):
                nc.sync.dma_start(embT[:, k, :],
                                  embT_d[:, N_TOK * k:N_TOK * (k + 1)])
            wih_enc_sb = wpool.tile([128, 4, 4096], MM_DT, tag="we")
            wih_lm_sb = wpool.tile([128, 4, 4096], MM_DT, tag="wl")
            for k in range(4):
                nc.sync.dma_start(wih_enc_sb[:, k, :],
                                  wih_enc_d[128 * k:128 * (k + 1), :])
                nc.sync.dma_start(wih_lm_sb[:, k, :],
                                  wih_lm_d[128 * k:128 * (k + 1), :])
            # encoder projections first (the encoder needs rows 0 and 63
            # immediately: fwd reads row s, bwd reads row 63-s)
            for blk in range(2):
                for s in range(32):
                    proj_unit(wih_enc_sb, tembT, 4, s, blk, b_enc_sb,
                              xp_enc_d[:, :, :], xpo)
            # LM projections (drain during the encoder recurrence)
            for blk in range(4):
                for s in range(32):
                    proj_unit(wih_lm_sb, embT, 4, s, blk, b_lm_sb,
                              xp_lm_d[:, :, :], xpo)

        # =========================================================
        # Recurrences
        # =========================================================
        with tc.tile_pool(name="state", bufs=2) as state, \
             tc.tile_pool(name="chain", bufs=2) as chp, \
             tc.tile_pool(name="xp", bufs=3) as xp_p, \
             tc.tile_pool(name="psum_g", bufs=2, space="PSUM") as psum_g:
            whh_enc_ctx = tc.tile_pool(name="whh_enc", bufs=1)
            whh_enc_p = whh_enc_ctx.__enter__()
            whh_enc = whh_enc_p.tile([128, 4, 4096], MM_DT)
            for k in range(4):
                nc.sync.dma_start(whh_enc[:, k, :],
                                  whh_enc_d[128 * k:128 * (k + 1), :])

            # ---- Phase C: encoder (fwd + bwd in one PSUM tile), 64 steps ----
            h_cur = state.tile([128, 128], MM_DT, tag="h")
            c_cur = state.tile([128, 128], MM_DT, tag="c")
            nc.sync.dma_start(h_cur[:], h0T_d[:])
            nc.sync.dma_start(c_cur[:], c0T_d[:])

            def enc_xt(s):
                xt = xp_p.tile([128, 512], MM_DT, tag="xt")
                nc.sync.dma_start(xt[:, 0:256], xp_enc_d[s, :, 0:256])
                nc.sync.dma_start(xt[:, 256:512], xp_enc_d[Lt - 1 - s, :, 256:512])
                return xt

            def enc_gates(pg, xt, h, ks, inject):
                if inject:
                    nc.tensor.matmul(pg[:], ident[:], xt[:], start=True,
                                     stop=False, skip_group_check=True)
                for s in range(32):
                    hoff = 64 if s >= 16 else 0
                    for k in ks:
                        nc.tensor.matmul(
                            pg[:, 16 * s:16 * (s + 1)],
                            whh_enc[:, k, 128 * s:128 * (s + 1)],
                            h[:, hoff + 16 * k:hoff + 16 * (k + 1)],
                            start=False, stop=(k == 3), skip_group_check=True)

            pg = psum_g.tile([128, 512], dt.float32, tag="pg")
            xt = enc_xt(0)
            enc_gates(pg, xt, h_cur, range(4), True)

            for st in range(Lt):
                last = st == Lt - 1
                sigb = chp.tile([128, 512], MM_DT, tag="sig")
                cn = state.tile([128, 128], MM_DT, tag="c")
                hn = state.tile([128, 128], MM_DT, tag="h")
                tcn = chp.tile([128, 128], MM_DT, tag="tcn")
                t1 = chp.tile([128, 128], MM_DT, tag="t1")
                t2 = chp.tile([128, 128], MM_DT, tag="t2")
                if not last:
                    pg_n = psum_g.tile([128, 512], dt.float32, tag="pg")
                    xt_n = enc_xt(st + 1)
                    nc.tensor.matmul(pg_n[:], ident[:], xt_n[:], start=True,
                                     stop=False, skip_group_check=True)
                for d, off, coff in ((0, 0, 0), (1, 256, 64)):
                    sg = slice(off, off + 192)
                    nc.scalar.activation(sigb[:, sg], pg[:, sg], ACT.Sigmoid)
                    gg = slice(off + 192, off + 256)
                    nc.scalar.activation(sigb[:, gg], pg[:, gg], ACT.Tanh)
                    cs = slice(coff, coff + 64)
                    nc.vector.tensor_mul(t1[:, cs], sigb[:, off + 64:off + 128],
                                         c_cur[:, cs])
                    nc.vector.tensor_mul(t2[:, cs], sigb[:, off:off + 64],
                                         sigb[:, gg])
                    nc.vector.tensor_add(cn[:, cs], t1[:, cs], t2[:, cs])
                    nc.scalar.activation(tcn[:, cs], cn[:, cs], ACT.Tanh)
                    nc.vector.tensor_mul(hn[:, cs], sigb[:, off + 128:off + 192],
                                         tcn[:, cs])
                    if not last:
                        ks = range(4)
                        for s in (range(0, 16) if d == 0 else range(16, 32)):
                            for k in ks:
                                nc.tensor.matmul(
                                    pg_n[:, 16 * s:16 * (s + 1)],
                                    whh_enc[:, k, 128 * s:128 * (s + 1)],
                                    hn[:, coff + 16 * k:coff + 16 * (k + 1)],
                                    start=False, stop=(k == 3),
                                    skip_group_check=True)
                c_cur, h_cur = cn, hn
                if not last:
                    pg, xt = pg_n, xt_n

            # ---- Phase D: reshuffle enc final states -> LM initial ----
            # h_lm[b, 128k+p] mixes batch (torch view(-1, 2He)):
            #   b<8:  src dir fwd, enc-row 2b + khalf;  b>=8: bwd, 2(b-8)+khalf
            c_lm = state.tile([128, 128], MM_DT, tag="c")
            for khalf in range(2):        # dst k<4 / k>=4  (even/odd src row)
                for bhalf in range(2):    # dst b<8 (fwd) / b>=8 (bwd)
                    for k in range(4):
                        s0 = 64 * bhalf + khalf + 16 * k
                        nc.sync.dma_start(
                            ysT[:, 4 * khalf + k, 8 * bhalf:8 * bhalf + 8],
                            h_cur[:, s0:s0 + 15:2])
                        d0 = 16 * (4 * khalf + k) + 8 * bhalf
                        nc.sync.dma_start(c_lm[:, d0:d0 + 8],
                                          c_cur[:, s0:s0 + 15:2])
            c_cur = c_lm

            whh_enc_ctx.__exit__(None, None, None)

            # ---- Phase E: LM recurrence, 128 steps, 2-chunk pipelined ----
            def lm_xt(t):
                xt = xp_p.tile([128, 512], MM_DT, tag="xt")
                nc.sync.dma_start(xt[:], xp_lm_d[t])
                return xt

            def lm_gates(pg, hslot, ks):
                for s in range(32):
                    for k in ks:
                        nc.tensor.matmul(
                            pg[:, 16 * s:16 * (s + 1)],
                            whh_lm[:, k, 128 * s:128 * (s + 1)],
                            ysT[:, k, 16 * hslot:16 * (hslot + 1)],
                            start=False, stop=(k == 7), skip_group_check=True)

            pg = psum_g.tile([128, 512], dt.float32, tag="pg")
            xt = lm_xt(0)
            nc.tensor.matmul(pg[:], ident[:], xt[:], start=True, stop=False,
                             skip_group_check=True)
            lm_gates(pg, 0, range(8))

            for t in range(T):
                last = t == T - 1
                sigb = chp.tile([128, 512], MM_DT, tag="sig")
                cn = state.tile([128, 128], MM_DT, tag="c")
                tcn = chp.tile([128, 128], MM_DT, tag="tcn")
                t1 = chp.tile([128, 128], MM_DT, tag="t1")
                t2 = chp.tile([128, 128], MM_DT, tag="t2")
                if not last:
                    pg_n = psum_g.tile([128, 512], dt.float32, tag="pg")
                    xt_n = lm_xt(t + 1)
                    nc.tensor.matmul(pg_n[:], ident[:], xt_n[:], start=True,
                                     stop=False, skip_group_check=True)
                # chunk A: k 0..3 (cols 0:64 of each gate region), chunk B: 4..7
                for ch in range(2):
                    co = 64 * ch
                    cs = slice(co, co + 64)
                    # sigmoid over i,f,o sub-cols of this chunk (strided AP)
                    pgv = pg[:, 0:384].rearrange("p (g c) -> p g c", g=3)
                    sgv = sigb[:, 0:384].rearrange("p (g c) -> p g c", g=3)
                    nc.scalar.activation(sgv[:, :, cs], pgv[:, :, cs], ACT.Sigmoid)
                    nc.scalar.activation(sigb[:, 384 + co:448 + co],
                                         pg[:, 384 + co:448 + co], ACT.Tanh)
                    nc.vector.tensor_mul(t1[:, cs], sigb[:, 128 + co:192 + co],
                                         c_cur[:, cs])
                    nc.vector.tensor_mul(t2[:, cs], sigb[:, co:64 + co],
                                         sigb[:, 384 + co:448 + co])
                    nc.vector.tensor_add(cn[:, cs], t1[:, cs], t2[:, cs])
                    nc.scalar.activation(tcn[:, cs], cn[:, cs], ACT.Tanh)
                    hv = ysT[:, 4 * ch:4 * (ch + 1), 16 * (t + 1):16 * (t + 2)]
                    nc.vector.tensor_mul(
                        hv, sigb[:, 256 + co:320 + co].rearrange(
                            "p (k b) -> p k b", k=4),
                        tcn[:, cs].rearrange("p (k b) -> p k b", k=4))
                    if not last:
                        lm_gates(pg_n, t + 1, range(4 * ch, 4 * ch + 4))
                c_cur = cn
                if not last:
                    pg, xt = pg_n, xt_n

        # =========================================================
        # Phase F: decode (vocab-major): out[128v, 512tok] per unit
        # =========================================================
        with tc.tile_pool(name="wdec", bufs=4) as wdp, \
             tc.tile_pool(name="dout", bufs=4) as dop:
            for sv in range(VSTRIPS):
                wn = wdp.tile([128, 8, 128], MM_DT, tag="wn")
                for k in range(8):
                    nc.sync.dma_start(
                        wn[:, k, :],
                        wdec_d[128 * k:128 * (k + 1), 128 * sv:128 * (sv + 1)])
                for blk in range(4):
                    pd = psum_mm.tile([128, 512], dt.float32, tag="mm")
                    for k in range(8):
                        nc.tensor.matmul(
                            pd[:], wn[:, k, :],
                            ysT[:, k, 16 + 512 * blk:16 + 512 * (blk + 1)],
                            start=(k == 0), stop=(k == 7))
                    ob = dop.tile([128, 512], MM_DT, tag="ob")
                    nc.scalar.activation(ob[:], pd[:], ACT.Identity,
                                         bias=bdec_sb[:, sv:sv + 1])
                    nc.gpsimd.dma_start(
                        out_d[128 * sv:128 * (sv + 1),
                              512 * blk:512 * (blk + 1)], ob[:])

    nc.compile()
    return nc


def _embT_host(tbl, idx):
    """Gather rows for flat token order r=16t+b, lay out [128, (E//128)*n]."""
    g = np.asarray(tbl, np.float32)[idx]            # [n, E]
    n = g.shape[0]
    gt = g.T.reshape(E // 128, 128, n)              # [k, p, n]
    return np.ascontiguousarray(
        gt.transpose(1, 0, 2).reshape(128, -1)).astype(MM_NP)


def _strip_bias(b):
    """[4H'] -> [128, nstrips] per-partition bias."""
    return np.ascontiguousarray(b.reshape(-1, 128).T).astype(np.float32)


def _prep_inputs(inputs):
    f32 = np.float32
    x = np.asarray(inputs["x"]).astype(np.int64)
    table = np.asarray(inputs["table"]).astype(np.int64)
    xf = x.T.reshape(-1)        # row r = 16t+b
    tf = table.T.reshape(-1)

    pe = _gate_perm(He)
    pl = _gate_perm(H)
    wih_enc_t = np.concatenate(
        [np.asarray(inputs["Wih_f"])[pe].T, np.asarray(inputs["Wih_b"])[pe].T],
        axis=1).astype(MM_NP)                       # [512, 4096]
    whh_enc_t = np.concatenate(
        [np.asarray(inputs["Whh_f"])[pe].T, np.asarray(inputs["Whh_b"])[pe].T],
        axis=1).astype(MM_NP)                       # [512, 4096]
    wih_lm_t = np.ascontiguousarray(np.asarray(inputs["Wih_lm"])[pl].T).astype(MM_NP)
    whh_lm_t = np.ascontiguousarray(np.asarray(inputs["Whh_lm"])[pl].T).astype(MM_NP)
    b_enc = np.concatenate([np.asarray(inputs["b_f"])[pe],
                            np.asarray(inputs["b_b"])[pe]])
    b_lm = np.asarray(inputs["b_lm"])[pl]

    # transposed initial enc states: cols = [fwd k0..3 | bwd k0..3] x 16
    def init_T(v):                                   # v: [2, B, He]
        v = np.asarray(v, f32)
        o = np.zeros((128, 128), f32)
        for d in range(2):
            for k in range(4):
                # o[p, 64d + 16k + b] = v[d, b, 128k+p]
                o[:, 64 * d + 16 * k:64 * d + 16 * (k + 1)] = \
                    v[d, :, 128 * k:128 * (k + 1)].T
        return o.astype(MM_NP)

    wdec = np.asarray(inputs["Wdec"]).astype(f32)
    bdec = np.asarray(inputs["bdec"]).astype(f32)
    wdec_pad = np.zeros((NCORES * VS, H), f32)
    wdec_pad[:V] = wdec
    bdec_pad = np.zeros(NCORES * VS, f32)
    bdec_pad[:V] = bdec

    common = dict(
        embT=_embT_host(inputs["embed"], xf),
        tembT=_embT_host(inputs["table_embed"], tf),
        wih_enc_t=wih_enc_t, wih_lm_t=wih_lm_t,
        whh_enc_t=whh_enc_t, whh_lm_t=whh_lm_t,
        b_enc_s=_strip_bias(b_enc), b_lm_s=_strip_bias(b_lm),
        h0T=init_T(inputs["enc_h0"]), c0T=init_T(inputs["enc_c0"]),
    )
    in_maps = []
    for c in range(NCORES):
        m = dict(common)
        m["wdec_t"] = np.ascontiguousarray(
            wdec_pad[c * VS:(c + 1) * VS].T).astype(MM_NP)
        m["bdec_s"] = _strip_bias(bdec_pad[c * VS:(c + 1) * VS])
        in_maps.append(m)
    return in_maps


def kernel(**inputs) -> np.ndarray:
    import time as _time
    if "nc" not in _CACHE:
        _CACHE["nc"] = build_bass()
    nc = _CACHE["nc"]
    in_maps = _prep_inputs(inputs)
    res = None
    for attempt in range(3):
        try:
            res = run_bass_kernel_spmd(nc, in_maps, core_ids=list(range(NCORES)))
            break
        except Exception:
            if attempt == 2:
                raise
            _time.sleep(10)
    outs = [np.asarray(res.results[c]["out"], np.float32) for c in range(NCORES)]
    full = np.concatenate(outs, axis=0)[:V]          # [V, 2048]
    return np.ascontiguousarray(full.T.reshape(T, B, V))


if __name__ == "__main__":
    nc = build_bass()
    print("build ok")
